# revision 7
# baseline (speedup 1.0000x reference)
"""Self-contained Trainium2 Bass kernel for a 6-layer post-LN transformer
encoder (B=2, S=2048, D=1024, H=16, F=4096, V=32000).

Sharding: sequence-parallel across 8 NeuronCores. Core c handles batch
b = c // 4, sequence slice [lc*512, (lc+1)*512) with lc = c % 4. Per layer,
one AllGather (replica groups [[0..3],[4..7]]) shares bf16 K/V across the
4 cores of each batch; all other compute is local to the core.

On-chip layout: residual stream is feature-major [1024 feats -> 8 chunks of
128 partitions, 512 tokens free] in f32. Matmul inputs are cast to bf16;
PSUM accumulation is f32. Attention scores are computed pre-transposed
(K @ Q^T per head) so softmax exp needs no transposes; the softmax
denominator comes from a ones-column appended to V in the AV matmul.
LayerNorm stats use ones-vector matmuls over the partition dim; rsqrt is
computed as exp(-0.5*ln(var+eps)) to stay in one ACT table set.
"""

import math
from contextlib import ExitStack

import ml_dtypes
import numpy as np

import concourse.bass as bass
import concourse.tile as tile
from concourse import bacc, mybir
from concourse.bass import IndirectOffsetOnAxis
from concourse.bass_utils import run_bass_kernel_spmd

dt = mybir.dt
Alu = mybir.AluOpType
Act = mybir.ActivationFunctionType

V, L, D, S, F, H, B = 32000, 6, 1024, 2048, 4096, 16, 2
DK = D // H          # 64
NC = 8               # cores
NT = (B * S) // NC   # 512 tokens per core
CH = D // 128        # 8 feature chunks
FCH = F // 128       # 32 ffn chunks
TC = NT // 128       # 4 token chunks per core
KCH = S // 128       # 16 key chunks per batch
EPS = 1e-5
SQRT_D = math.sqrt(D)
KV_HALF = D * NT     # flat elems of K (or V) contribution per core


def _ln(ctx, nc, psum, xres, xbfp, stats, x_in, g_sb, be_sb, ones_col_bf,
        ones_row_bf, eps_sb, name):
    """LayerNorm over features (partition dim) of feature-major x_in
    [128, CH, 512] f32. Returns (x_out f32 tile, x_out bf16 tile)."""
    # bf16 copies for stats matmuls, in a rotating 2-chunk scratch;
    # column sums via ones-matmuls accumulated over feature chunks
    x_b = xbfp.tile([128, 2, NT], dt.bfloat16, tag="sq", bufs=1,
                    name=f"xb_{name}")
    x_sq = xbfp.tile([128, 2, NT], dt.bfloat16, tag="sq2", bufs=1,
                     name=f"xsq_{name}")
    st_sum = psum.tile([1, NT], dt.float32, tag="acc", name=f"sts_{name}")
    st_sq = psum.tile([1, NT], dt.float32, tag="acc", name=f"stq_{name}")
    for kc in range(CH):
        s = kc % 2
        nc.vector.tensor_copy(out=x_b[:, s, :], in_=x_in[:, kc, :])
        nc.vector.tensor_tensor(out=x_sq[:, s, :], in0=x_b[:, s, :],
                                in1=x_b[:, s, :], op=Alu.mult)
        nc.tensor.matmul(st_sum[:, :], lhsT=ones_col_bf[:, :],
                         rhs=x_b[:, s, :], start=(kc == 0),
                         stop=(kc == CH - 1), skip_group_check=True)
        nc.tensor.matmul(st_sq[:, :], lhsT=ones_col_bf[:, :],
                         rhs=x_sq[:, s, :], start=(kc == 0),
                         stop=(kc == CH - 1), skip_group_check=True)
    # [1, *] scalar lane: mean, var, rs = exp(-0.5*ln(var+eps)), mu*rs
    sm = stats.tile([1, 2, NT], dt.float32, tag="sm", name=f"sm_{name}")
    nc.vector.tensor_copy(out=sm[:, 0, :], in_=st_sum[:, :])
    nc.vector.tensor_copy(out=sm[:, 1, :], in_=st_sq[:, :])
    nc.vector.tensor_scalar(out=sm[:, :, :], in0=sm[:, :, :],
                            scalar1=1.0 / D, scalar2=None, op0=Alu.mult)
    var = stats.tile([1, NT], dt.float32, tag="var", name=f"var_{name}")
    nc.vector.tensor_tensor(out=var[:, :], in0=sm[:, 0, :], in1=sm[:, 0, :],
                            op=Alu.mult)
    nc.vector.tensor_tensor(out=var[:, :], in0=sm[:, 1, :], in1=var[:, :],
                            op=Alu.subtract)
    # rsmu = [rs | mu*rs] on partition 0 (bf16: feeds the broadcast matmul)
    rsmu = stats.tile([1, 2, NT], dt.bfloat16, tag="rsmu", name=f"rsmu_{name}")
    nc.scalar.activation(out=var[:, :], in_=var[:, :], func=Act.Ln,
                         bias=eps_sb[:, :], scale=1.0)
    nc.scalar.activation(out=rsmu[:, 0, :], in_=var[:, :], func=Act.Exp,
                         bias=0.0, scale=-0.5)
    nc.vector.tensor_tensor(out=rsmu[:, 1, :], in0=sm[:, 0, :],
                            in1=rsmu[:, 0, :], op=Alu.mult)
    # broadcast rs and mu*rs across partitions via K=1 fp32 matmul
    bc = psum.tile([128, 2, NT], dt.float32, tag="sc", bufs=1,
                   name=f"bc_{name}")
    nc.tensor.matmul(bc[:, 0, :], lhsT=ones_row_bf[:, :], rhs=rsmu[:, 0, :],
                     start=True, stop=True)
    nc.tensor.matmul(bc[:, 1, :], lhsT=ones_row_bf[:, :], rhs=rsmu[:, 1, :],
                     start=True, stop=True)
    # apply per chunk: x_out = (x*rs - mu*rs)*g + be, then bf16 cast.
    # Chunked ops let the next phase's matmuls start on early chunks via
    # Tile's subtile dependencies instead of waiting for the full tile.
    t = xres.tile([128, CH, NT], dt.float32, tag="x", name=f"t_{name}")
    x_out = xres.tile([128, CH, NT], dt.float32, tag="x", name=f"xo_{name}")
    x_out_bf = xbfp.tile([128, CH, NT], dt.bfloat16, tag="xb", bufs=1,
                         name=f"xob_{name}")
    for c in range(CH):
        nc.vector.tensor_tensor(out=t[:, c, :], in0=x_in[:, c, :],
                                in1=bc[:, 0, :], op=Alu.mult)
        nc.vector.tensor_tensor(out=t[:, c, :], in0=t[:, c, :],
                                in1=bc[:, 1, :], op=Alu.subtract)
        nc.vector.tensor_scalar(out=x_out[:, c, :], in0=t[:, c, :],
                                scalar1=g_sb[:, c:c + 1],
                                scalar2=be_sb[:, c:c + 1],
                                op0=Alu.mult, op1=Alu.add)
        nc.vector.tensor_copy(out=x_out_bf[:, c, :], in_=x_out[:, c, :])
    return x_out, x_out_bf


def build_program(layers=L):
    nc = bacc.Bacc("TRN2", target_bir_lowering=False, debug=False,
                   num_devices=NC)
    # ---------------- I/O ----------------
    tok_d = nc.dram_tensor("tokens_c", [NT], dt.int32, kind="ExternalInput")
    emb_d = nc.dram_tensor("emb", [V, D], dt.float32, kind="ExternalInput")
    pe_d = nc.dram_tensor("pe_fm", [D, NT], dt.float32, kind="ExternalInput")
    wq_d = nc.dram_tensor("Wq", [layers, D, D], dt.bfloat16, kind="ExternalInput")
    wk_d = nc.dram_tensor("Wk", [layers, D, D], dt.bfloat16, kind="ExternalInput")
    wv_d = nc.dram_tensor("Wv", [layers, D, D], dt.bfloat16, kind="ExternalInput")
    wo_d = nc.dram_tensor("Wo", [layers, D, D], dt.bfloat16, kind="ExternalInput")
    w1_d = nc.dram_tensor("W1", [layers, D, F], dt.bfloat16, kind="ExternalInput")
    w2_d = nc.dram_tensor("W2", [layers, F, D], dt.bfloat16, kind="ExternalInput")
    bq_d = nc.dram_tensor("bq", [layers, D], dt.float32, kind="ExternalInput")
    bk_d = nc.dram_tensor("bk", [layers, D], dt.float32, kind="ExternalInput")
    bv_d = nc.dram_tensor("bv", [layers, D], dt.float32, kind="ExternalInput")
    bo_d = nc.dram_tensor("bo", [layers, D], dt.float32, kind="ExternalInput")
    b1_d = nc.dram_tensor("b1", [layers, F], dt.float32, kind="ExternalInput")
    b2_d = nc.dram_tensor("b2", [layers, D], dt.float32, kind="ExternalInput")
    g1_d = nc.dram_tensor("g1", [layers, D], dt.float32, kind="ExternalInput")
    be1_d = nc.dram_tensor("be1", [layers, D], dt.float32, kind="ExternalInput")
    g2_d = nc.dram_tensor("g2", [layers, D], dt.float32, kind="ExternalInput")
    be2_d = nc.dram_tensor("be2", [layers, D], dt.float32, kind="ExternalInput")
    out_d = nc.dram_tensor("out_fm", [D, NT], dt.float32, kind="ExternalOutput")

    groups = [[0, 1, 2, 3], [4, 5, 6, 7]]

    with tile.TileContext(nc) as tc, ExitStack() as ctx:
        psum = ctx.enter_context(tc.tile_pool(name="psum", bufs=2, space="PSUM"))
        consts = ctx.enter_context(tc.tile_pool(name="consts", bufs=1))
        small = ctx.enter_context(tc.tile_pool(name="small", bufs=1))
        xres = ctx.enter_context(tc.tile_pool(name="xres", bufs=3))
        xbfp = ctx.enter_context(tc.tile_pool(name="xbf", bufs=2))
        qkvp = ctx.enter_context(tc.tile_pool(name="qkv", bufs=1))
        attnp = ctx.enter_context(tc.tile_pool(name="attn", bufs=4))
        kvsp = ctx.enter_context(tc.tile_pool(name="kvs", bufs=2))
        wpool = ctx.enter_context(tc.tile_pool(name="w", bufs=3))
        hpool = ctx.enter_context(tc.tile_pool(name="h", bufs=1))
        opool = ctx.enter_context(tc.tile_pool(name="o", bufs=1))
        stats = ctx.enter_context(tc.tile_pool(name="stats", bufs=1))
        dram = ctx.enter_context(tc.tile_pool(name="dram", bufs=2, space="DRAM"))

        # ---------------- constants ----------------
        ident = consts.tile([128, 128], dt.float32, name="ident")
        from concourse.masks import make_identity
        make_identity(nc, ident[:, :])
        ones_col_bf = consts.tile([128, 1], dt.bfloat16, name="ones_col")
        nc.vector.memset(ones_col_bf[:, :], 1.0)
        ones_row_bf = consts.tile([1, 128], dt.bfloat16, name="ones_row")
        nc.vector.memset(ones_row_bf[:, :], 1.0)
        eps_sb = consts.tile([1, 1], dt.float32, name="eps_sb")
        nc.vector.memset(eps_sb[:, :], EPS)

        # ---------------- embedding ----------------
        tok_sb = consts.tile([128, TC], dt.int32, name="tok_sb")
        nc.sync.dma_start(out=tok_sb[:, :],
                          in_=tok_d[:].rearrange("(c p) -> p c", p=128))
        pe_sb = xres.tile([128, CH, NT], dt.float32, tag="x", name="pe_sb")
        nc.sync.dma_start(out=pe_sb[:, :, :],
                          in_=pe_d[:, :].rearrange("(c p) q -> p c q", p=128))
        x = xres.tile([128, CH, NT], dt.float32, tag="x", name="x0")
        for tc4 in range(TC):
            gath = wpool.tile([128, D], dt.float32, tag="w", name=f"gath{tc4}")
            nc.gpsimd.indirect_dma_start(
                out=gath[:, :], out_offset=None, in_=emb_d[:, :],
                in_offset=IndirectOffsetOnAxis(ap=tok_sb[:, tc4:tc4 + 1], axis=0))
            for fc in range(CH):
                tp = psum.tile([128, NT], dt.float32, tag="acc",
                               name=f"tp{tc4}_{fc}")
                nc.tensor.transpose(tp[:, 0:128],
                                    gath[:, fc * 128:(fc + 1) * 128],
                                    ident[:, :])
                nc.vector.tensor_scalar(
                    out=x[:, fc, tc4 * 128:(tc4 + 1) * 128], in0=tp[:, 0:128],
                    scalar1=float(SQRT_D), scalar2=None, op0=Alu.mult)
        nc.vector.tensor_tensor(out=x[:, :, :], in0=x[:, :, :],
                                in1=pe_sb[:, :, :], op=Alu.add)
        x_bf = xbfp.tile([128, CH, NT], dt.bfloat16, tag="xb", bufs=1,
                         name="x0_bf")
        nc.vector.tensor_copy(out=x_bf[:, :, :], in_=x[:, :, :])

        # ---------------- layers ----------------
        for l in range(layers):
            wq_r = wq_d[l].rearrange("(kc p) f -> p kc f", p=128)
            wk_r = wk_d[l].rearrange("(kc p) f -> p kc f", p=128)
            wv_r = wv_d[l].rearrange("(kc p) f -> p kc f", p=128)
            wo_r = wo_d[l].rearrange("(kc p) f -> p kc f", p=128)
            w1_r = w1_d[l].rearrange("(kc p) f -> p kc f", p=128)
            w2_r = w2_d[l].rearrange("(kc p) f -> p kc f", p=128)

            bq_sb = small.tile([128, CH], dt.float32, tag="bq", name=f"bq{l}")
            nc.sync.dma_start(out=bq_sb[:, :],
                              in_=bq_d[l].rearrange("(c p) -> p c", p=128))
            bk_sb = small.tile([128, CH], dt.float32, tag="bk", name=f"bk{l}")
            nc.sync.dma_start(out=bk_sb[:, :],
                              in_=bk_d[l].rearrange("(c p) -> p c", p=128))
            bo_sb = small.tile([128, CH], dt.float32, tag="bo", name=f"bo{l}")
            nc.sync.dma_start(out=bo_sb[:, :],
                              in_=bo_d[l].rearrange("(c p) -> p c", p=128))
            b2_sb = small.tile([128, CH], dt.float32, tag="b2", name=f"b2{l}")
            nc.sync.dma_start(out=b2_sb[:, :],
                              in_=b2_d[l].rearrange("(c p) -> p c", p=128))
            b1_sb = small.tile([128, FCH], dt.float32, tag="b1", name=f"b1{l}")
            nc.sync.dma_start(out=b1_sb[:, :],
                              in_=b1_d[l].rearrange("(c p) -> p c", p=128))
            g1_sb = small.tile([128, CH], dt.float32, tag="g1", name=f"g1{l}")
            nc.sync.dma_start(out=g1_sb[:, :],
                              in_=g1_d[l].rearrange("(c p) -> p c", p=128))
            be1_sb = small.tile([128, CH], dt.float32, tag="be1", name=f"be1{l}")
            nc.sync.dma_start(out=be1_sb[:, :],
                              in_=be1_d[l].rearrange("(c p) -> p c", p=128))
            g2_sb = small.tile([128, CH], dt.float32, tag="g2", name=f"g2{l}")
            nc.sync.dma_start(out=g2_sb[:, :],
                              in_=g2_d[l].rearrange("(c p) -> p c", p=128))
            be2_sb = small.tile([128, CH], dt.float32, tag="be2", name=f"be2{l}")
            nc.sync.dma_start(out=be2_sb[:, :],
                              in_=be2_d[l].rearrange("(c p) -> p c", p=128))
            bv_l = bv_d[l]
            bv_bc = small.tile([128, D], dt.float32, tag="bv", name=f"bv{l}")
            nc.gpsimd.dma_start(
                out=bv_bc[:, :],
                in_=bass.AP(tensor=bv_l.tensor, offset=bv_l.offset,
                            ap=[[0, 128]] + list(bv_l.ap)))

            # ---- K/V projections, each half kicking its AllGather early so
            # the collectives pipeline under Q projection + attention ----
            # K is feature-major [1024, NT]; half A = heads 0-7 (chunks 0-3).
            # V is token-major [NT, 1024]; half A = head-feats 0-511.
            k_bf = qkvp.tile([128, CH, NT], dt.bfloat16, tag="k", name=f"k{l}")
            v_bf = qkvp.tile([128, TC, D], dt.bfloat16, tag="v", name=f"v{l}")
            kg_half, vg_half = [], []

            def k_half(hf):
                for m in range(4 * hf, 4 * hf + 4):
                    wm = wpool.tile([128, CH, 128], dt.bfloat16, tag="w",
                                    name=f"wk{l}_{m}")
                    nc.sync.dma_start(out=wm[:, :, :],
                                      in_=wk_r[:, :, m * 128:(m + 1) * 128])
                    ps = psum.tile([128, NT], dt.float32, tag="acc",
                                   name=f"psk{l}_{m}")
                    for kc in range(CH):
                        nc.tensor.matmul(ps[:, :], lhsT=wm[:, kc, :],
                                         rhs=x_bf[:, kc, :],
                                         start=(kc == 0), stop=(kc == CH - 1))
                    nc.vector.tensor_scalar(out=k_bf[:, m, :], in0=ps[:, :],
                                            scalar1=bk_sb[:, m:m + 1],
                                            scalar2=None, op0=Alu.add)
                own = dram.tile([KV_HALF // 2], dt.bfloat16, tag=f"ko{hf}",
                                name=f"ko{l}_{hf}")
                nc.sync.dma_start(
                    out=own[:].rearrange("(c p q) -> p c q", p=128, q=NT),
                    in_=k_bf[:, 4 * hf:4 * hf + 4, :])
                gth = dram.tile([4, KV_HALF // 2], dt.bfloat16, tag=f"kg{hf}",
                                name=f"kg{l}_{hf}")
                nc.gpsimd.collective_compute(
                    "AllGather", Alu.bypass, replica_groups=groups,
                    ins=[own[:].opt()], outs=[gth[:, :].opt()])
                kg_half.append(gth[:, :].rearrange(
                    "g (c p q) -> c p g q", c=4, p=128, q=NT))

            def v_half(hf):
                wvn = wpool.tile([128, CH, 512], dt.bfloat16, tag="w",
                                 name=f"wv{l}_{hf}")
                nc.sync.dma_start(out=wvn[:, :, :],
                                  in_=wv_r[:, :, hf * 512:(hf + 1) * 512])
                for tc4 in range(TC):
                    ps = psum.tile([128, 512], dt.float32, tag="acc",
                                   name=f"psv{l}_{hf}_{tc4}")
                    for kc in range(CH):
                        nc.tensor.matmul(
                            ps[:, :],
                            lhsT=x_bf[:, kc, tc4 * 128:(tc4 + 1) * 128],
                            rhs=wvn[:, kc, :],
                            start=(kc == 0), stop=(kc == CH - 1))
                    nc.vector.tensor_tensor(
                        out=v_bf[:, tc4, hf * 512:(hf + 1) * 512], in0=ps[:, :],
                        in1=bv_bc[:, hf * 512:(hf + 1) * 512], op=Alu.add)
                own = dram.tile([KV_HALF // 2], dt.bfloat16, tag=f"vo{hf}",
                                name=f"vo{l}_{hf}")
                nc.sync.dma_start(
                    out=own[:].rearrange("(t p f) -> p t f", p=128, f=512),
                    in_=v_bf[:, :, hf * 512:(hf + 1) * 512])
                gth = dram.tile([4, KV_HALF // 2], dt.bfloat16, tag=f"vg{hf}",
                                name=f"vg{l}_{hf}")
                nc.gpsimd.collective_compute(
                    "AllGather", Alu.bypass, replica_groups=groups,
                    ins=[own[:].opt()], outs=[gth[:, :].opt()])
                vg_half.append(gth[:, :].rearrange(
                    "g (t p f) -> p (g t) f", p=128, f=512))

            k_half(0)
            v_half(0)
            k_half(1)
            v_half(1)

            # ---- Q projection (feature-major; head pair per chunk) ----
            q_bf = qkvp.tile([128, CH, NT], dt.bfloat16, tag="q", name=f"q{l}")
            for m in range(CH):
                wm = wpool.tile([128, CH, 128], dt.bfloat16, tag="w",
                                name=f"wq{l}_{m}")
                nc.sync.dma_start(out=wm[:, :, :],
                                  in_=wq_r[:, :, m * 128:(m + 1) * 128])
                ps = psum.tile([128, NT], dt.float32, tag="acc",
                               name=f"psq{l}_{m}")
                for kc in range(CH):
                    nc.tensor.matmul(ps[:, :], lhsT=wm[:, kc, :],
                                     rhs=x_bf[:, kc, :],
                                     start=(kc == 0), stop=(kc == CH - 1))
                nc.vector.tensor_scalar(out=q_bf[:, m, :], in0=ps[:, :],
                                        scalar1=bq_sb[:, m:m + 1],
                                        scalar2=None, op0=Alu.add)

            # ---- attention: one head PAIR per feature chunk; the two heads'
            # scores matmuls row-pack the PE array (K=64 each). The kc loop
            # runs scores+exp LAG steps ahead of the AV accumulation so the
            # Scalar exp stream (the phase's rate limiter) never waits on the
            # PE and vice versa. Wo runs as two half-passes interleaved into
            # the stream as PE filler, accumulating into x1a / x1. ----
            LAG = 4
            o_nbf = opool.tile([128, CH, NT], dt.bfloat16, tag="onb",
                               name=f"onb{l}")
            x1a = xres.tile([128, CH, NT], dt.bfloat16, tag="x1a", bufs=1,
                            name=f"x1a_{l}")
            x1 = xres.tile([128, CH, NT], dt.float32, tag="x", name=f"x1_{l}")
            filler = []

            def emit_filler(n):
                for _ in range(min(n, len(filler))):
                    filler.pop(0)()

            def wo_pass(half, wo_sb):
                # one output chunk m per filler slot: 4 accumulation matmuls
                # + one DVE op folding into x1a (pass A) or x1 (pass B)
                def mk(m):
                    def th():
                        ps = psum.tile([128, NT], dt.float32, tag="acc",
                                       name=f"pso{l}_{half}_{m}")
                        for kc in range(4):
                            nc.tensor.matmul(
                                ps[:, :], lhsT=wo_sb[:, kc, m * 128:(m + 1) * 128],
                                rhs=o_nbf[:, 4 * half + kc, :],
                                start=(kc == 0), stop=(kc == 3))
                        if half == 0:
                            nc.vector.tensor_scalar(out=x1a[:, m, :],
                                                    in0=ps[:, :],
                                                    scalar1=bo_sb[:, m:m + 1],
                                                    scalar2=None, op0=Alu.add)
                        else:
                            nc.vector.tensor_tensor(out=x1[:, m, :],
                                                    in0=ps[:, :],
                                                    in1=x1a[:, m, :],
                                                    op=Alu.add)
                            nc.vector.tensor_tensor(out=x1[:, m, :],
                                                    in0=x1[:, m, :],
                                                    in1=x[:, m, :], op=Alu.add)
                    return th
                for m in range(CH):
                    filler.append(mk(m))

            wo_sbs = []
            for c in range(CH):
                kga = kg_half[c // 4]
                k_sb = kvsp.tile([128, 4, NT], dt.bfloat16, tag="k",
                                 name=f"ksb{l}_{c}")
                nc.sync.dma_start(out=k_sb[:, :, :], in_=kga[c % 4])
                vsbs = []
                for i in range(2):
                    h = 2 * c + i
                    vga = vg_half[h // 8]
                    v_sb = kvsp.tile([128, KCH, 65], dt.bfloat16, tag=f"v{i}",
                                     name=f"vsb{l}_{h}")
                    nc.vector.memset(v_sb[:, :, 64:65], 1.0)
                    nc.sync.dma_start(
                        out=v_sb[:, :, 0:64],
                        in_=vga[:, :, (h % 8) * 64:(h % 8) * 64 + 64])
                    vsbs.append(v_sb)
                avs = [psum.tile([128, NT], dt.float32, tag="av",
                                 name=f"av{l}_{2 * c + i}") for i in range(2)]
                sc = psum.tile([128, 2, 2, NT], dt.float32, tag="sc", bufs=1,
                               name=f"sc{l}_{c}")
                ats = {}
                for kc in range(KCH + LAG):
                    if kc < KCH:
                        for i in range(2):
                            nc.tensor.matmul(
                                sc[:, kc % 2, i, :],
                                lhsT=k_sb[64 * i:64 * i + 64, kc // 4,
                                          (kc % 4) * 128:(kc % 4 + 1) * 128],
                                rhs=q_bf[64 * i:64 * i + 64, c, :],
                                start=True, stop=True,
                                tile_position=(64 * i, 0))
                        at = attnp.tile([128, 2, NT], dt.bfloat16, tag="at",
                                        bufs=6, name=f"at{l}_{c}_{kc}")
                        nc.scalar.activation(out=at[:, :, :],
                                             in_=sc[:, kc % 2, :, :],
                                             func=Act.Exp, bias=0.0,
                                             scale=0.125)
                        ats[kc] = at
                    if kc >= LAG:
                        j = kc - LAG
                        at = ats.pop(j)
                        for i in range(2):
                            nc.tensor.matmul(avs[i][0:65, :],
                                             lhsT=vsbs[i][:, j, :],
                                             rhs=at[:, i, :], start=(j == 0),
                                             stop=(j == KCH - 1),
                                             skip_group_check=True)
                        emit_filler(1)
                # normalize by the softmax denominator (row 64 of av); this
                # executes under the next chunk's scores/exp stream.
                for i in range(2):
                    h = 2 * c + i
                    av = avs[i]
                    o_h = attnp.tile([64, NT], dt.bfloat16, tag="oh", bufs=2,
                                     name=f"oh{l}_{h}")
                    nc.vector.tensor_copy(out=o_h[:, :], in_=av[0:64, :])
                    z_h = stats.tile([1, NT], dt.float32, tag="zh",
                                     name=f"zh{l}_{h}")
                    nc.vector.tensor_copy(out=z_h[:, :], in_=av[64:65, :])
                    zi_f = stats.tile([1, NT], dt.float32, tag="zif",
                                      name=f"zif{l}_{h}")
                    nc.vector.reciprocal_approx_fast(out=zi_f[:, :],
                                                     in_=z_h[:, :])
                    zi_h = stats.tile([1, NT], dt.bfloat16, tag="zi",
                                      name=f"zi{l}_{h}")
                    with nc.allow_low_precision(reason="1/Z feeds a bf16 "
                                                "broadcast matmul anyway"):
                        nc.vector.tensor_copy(out=zi_h[:, :], in_=zi_f[:, :])
                    bz = psum.tile([64, NT], dt.float32, tag="acc",
                                   name=f"bz{l}_{h}")
                    nc.tensor.matmul(bz[:, :], lhsT=ones_row_bf[0:1, 0:64],
                                     rhs=zi_h[:, :], start=True, stop=True)
                    nc.vector.tensor_tensor(out=o_nbf[64 * i:64 * i + 64, c, :],
                                            in0=o_h[:, :], in1=bz[:, :],
                                            op=Alu.mult)
                # prefetch the Wo weight half needed one chunk later, then
                # queue its 8 output-chunk thunks as PE filler
                if c in (2, 6):
                    half = (c - 2) // 4
                    wo_sb = wpool.tile([128, 4, D], dt.bfloat16, tag="wo",
                                       bufs=1, name=f"wo{l}_{half}")
                    nc.sync.dma_start(out=wo_sb[:, :, :],
                                      in_=wo_r[:, 4 * half:4 * half + 4, :])
                    wo_sbs.append(wo_sb)
                if c in (3, 7):
                    wo_pass((c - 3) // 4, wo_sbs[(c - 3) // 4])
            emit_filler(len(filler))

            x1n, x1n_bf = _ln(ctx, nc, psum, xres, xbfp, stats, x1, g1_sb,
                              be1_sb, ones_col_bf, ones_row_bf, eps_sb,
                              f"l{l}a")

            # ---- FFN ----
            x2 = xres.tile([128, CH, NT], dt.float32, tag="x", name=f"x2_{l}")
            for half in range(2):
                h_bf = hpool.tile([128, FCH // 2, NT], dt.bfloat16, tag="h",
                                  name=f"h{l}_{half}")
                for mi in range(FCH // 2):
                    m = half * (FCH // 2) + mi
                    w1m = wpool.tile([128, CH, 128], dt.bfloat16, tag="w",
                                     name=f"w1_{l}_{m}")
                    nc.sync.dma_start(out=w1m[:, :, :],
                                      in_=w1_r[:, :, m * 128:(m + 1) * 128])
                    ps = psum.tile([128, NT], dt.float32, tag="acc",
                                   name=f"ps1{l}_{m}")
                    for kc in range(CH):
                        nc.tensor.matmul(ps[:, :], lhsT=w1m[:, kc, :],
                                         rhs=x1n_bf[:, kc, :],
                                         start=(kc == 0), stop=(kc == CH - 1))
                    nc.vector.tensor_scalar(out=h_bf[:, mi, :], in0=ps[:, :],
                                            scalar1=b1_sb[:, m:m + 1],
                                            scalar2=0.0, op0=Alu.add,
                                            op1=Alu.max)
                for m in range(CH):
                    w2m = wpool.tile([128, FCH // 2, 128], dt.bfloat16, tag="w",
                                     name=f"w2_{l}_{half}_{m}")
                    nc.sync.dma_start(
                        out=w2m[:, :, :],
                        in_=w2_r[:, half * (FCH // 2):(half + 1) * (FCH // 2),
                                 m * 128:(m + 1) * 128])
                    ps = psum.tile([128, NT], dt.float32, tag="acc",
                                   name=f"ps2{l}_{half}_{m}")
                    for kc in range(FCH // 2):
                        nc.tensor.matmul(ps[:, :], lhsT=w2m[:, kc, :],
                                         rhs=h_bf[:, kc, :],
                                         start=(kc == 0),
                                         stop=(kc == FCH // 2 - 1))
                    if half == 0:
                        nc.vector.tensor_scalar(out=x2[:, m, :], in0=ps[:, :],
                                                scalar1=b2_sb[:, m:m + 1],
                                                scalar2=None, op0=Alu.add)
                    else:
                        nc.vector.tensor_tensor(out=x2[:, m, :],
                                                in0=x2[:, m, :], in1=ps[:, :],
                                                op=Alu.add)
                        nc.vector.tensor_tensor(out=x2[:, m, :],
                                                in0=x2[:, m, :],
                                                in1=x1n[:, m, :], op=Alu.add)

            x, x_bf = _ln(ctx, nc, psum, xres, xbfp, stats, x2, g2_sb, be2_sb,
                          ones_col_bf, ones_row_bf, eps_sb, f"l{l}b")

        # ---------------- output ----------------
        nc.sync.dma_start(
            out=out_d[:, :].rearrange("(c p) q -> p c q", p=128),
            in_=x[:, :, :])

    nc.compile()
    return nc


_PROG = {}


def _get_prog(layers=L):
    if layers not in _PROG:
        _PROG[layers] = build_program(layers)
    return _PROG[layers]


def _host_inputs(inputs, layers=L):
    """Build the 8 per-core input maps from the full-model inputs."""
    bf16 = ml_dtypes.bfloat16
    f32 = np.float32
    tokens = np.asarray(inputs["tokens"])
    # positional encoding (constant)
    pos = np.arange(S)[:, None].astype(f32)
    freq = np.exp(np.arange(0, D, 2).astype(f32) * -(math.log(10000.0) / D))
    pe = np.zeros((S, D), dtype=f32)
    pe[:, 0::2] = np.sin(pos * freq)
    pe[:, 1::2] = np.cos(pos * freq)
    shared = {
        "emb": np.ascontiguousarray(np.asarray(inputs["emb"], dtype=f32)),
    }
    for nm in ("Wq", "Wk", "Wv", "Wo", "W1", "W2"):
        shared[nm] = np.ascontiguousarray(
            np.asarray(inputs[nm])[:layers].astype(bf16))
    for nm in ("bq", "bk", "bv", "bo", "b1", "b2", "g1", "be1", "g2", "be2"):
        shared[nm] = np.ascontiguousarray(
            np.asarray(inputs[nm])[:layers].astype(f32))

    in_maps = []
    for core in range(NC):
        b, lc = core // 4, core % 4
        m = dict(shared)
        m["tokens_c"] = np.ascontiguousarray(tokens[b, lc * NT:(lc + 1) * NT])
        m["pe_fm"] = np.ascontiguousarray(pe[lc * NT:(lc + 1) * NT, :].T)
        in_maps.append(m)
    return in_maps


def run(inputs, layers=L, trace=False):
    nc = _get_prog(layers)
    in_maps = _host_inputs(inputs, layers)
    last_err = None
    for attempt in range(3):
        try:
            res = run_bass_kernel_spmd(nc, in_maps, list(range(NC)),
                                       trace=trace)
            break
        except Exception as e:  # transient NRT device faults observed
            last_err = e
            if attempt == 2:
                raise
            import time
            time.sleep(5)
    out = np.zeros((B, S, D), dtype=np.float32)
    for core in range(NC):
        b, lc = core // 4, core % 4
        out[b, lc * NT:(lc + 1) * NT, :] = res.results[core]["out_fm"].T
    return out, res


def kernel(**inputs):
    out, _ = run(inputs)
    return out



# revision 13
# speedup vs baseline: 1.1058x; 1.1058x over previous
"""Self-contained Trainium2 Bass kernel for a 6-layer post-LN transformer
encoder (B=2, S=2048, D=1024, H=16, F=4096, V=32000).

Sharding: sequence-parallel across 8 NeuronCores. Core c handles batch
b = c // 4, sequence slice [lc*512, (lc+1)*512) with lc = c % 4. Per layer,
one AllGather (replica groups [[0..3],[4..7]]) shares bf16 K/V across the
4 cores of each batch; all other compute is local to the core.

On-chip layout: residual stream is feature-major [1024 feats -> 8 chunks of
128 partitions, 512 tokens free] in f32. Matmul inputs are cast to bf16;
PSUM accumulation is f32. Attention scores are computed pre-transposed
(K @ Q^T per head) so softmax exp needs no transposes; the softmax
denominator comes from a ones-column appended to V in the AV matmul.
LayerNorm stats use ones-vector matmuls over the partition dim; rsqrt is
computed as exp(-0.5*ln(var+eps)) to stay in one ACT table set.
"""

import math
from contextlib import ExitStack

import ml_dtypes
import numpy as np

import concourse.bass as bass
import concourse.tile as tile
from concourse import bacc, mybir
from concourse.bass import IndirectOffsetOnAxis
from concourse.bass_utils import run_bass_kernel_spmd

dt = mybir.dt
Alu = mybir.AluOpType
Act = mybir.ActivationFunctionType

V, L, D, S, F, H, B = 32000, 6, 1024, 2048, 4096, 16, 2
DK = D // H          # 64
NC = 8               # cores
NT = (B * S) // NC   # 512 tokens per core
CH = D // 128        # 8 feature chunks
FCH = F // 128       # 32 ffn chunks
TC = NT // 128       # 4 token chunks per core
KCH = S // 128       # 16 key chunks per batch
EPS = 1e-5
SQRT_D = math.sqrt(D)
KV_HALF = D * NT     # flat elems of K (or V) contribution per core


def _ln(ctx, nc, psum, xres, xbfp, stats, x_in, g_sb, be_sb, ones_col_bf,
        ones_row_bf, eps_sb, name):
    """LayerNorm over features (partition dim) of feature-major x_in
    [128, CH, 512] f32. Returns (x_out f32 tile, x_out bf16 tile)."""
    # bf16 copies for stats matmuls, in a rotating 2-chunk scratch;
    # column sums via ones-matmuls accumulated over feature chunks
    x_b = xbfp.tile([128, 2, NT], dt.bfloat16, tag="sq", bufs=1,
                    name=f"xb_{name}")
    x_sq = xbfp.tile([128, 2, NT], dt.bfloat16, tag="sq2", bufs=1,
                     name=f"xsq_{name}")
    st_sum = psum.tile([1, NT], dt.float32, tag="acc", name=f"sts_{name}")
    st_sq = psum.tile([1, NT], dt.float32, tag="acc", name=f"stq_{name}")
    for kc in range(CH):
        s = kc % 2
        nc.vector.tensor_copy(out=x_b[:, s, :], in_=x_in[:, kc, :])
        nc.vector.tensor_tensor(out=x_sq[:, s, :], in0=x_b[:, s, :],
                                in1=x_b[:, s, :], op=Alu.mult)
        nc.tensor.matmul(st_sum[:, :], lhsT=ones_col_bf[:, :],
                         rhs=x_b[:, s, :], start=(kc == 0),
                         stop=(kc == CH - 1), skip_group_check=True)
        nc.tensor.matmul(st_sq[:, :], lhsT=ones_col_bf[:, :],
                         rhs=x_sq[:, s, :], start=(kc == 0),
                         stop=(kc == CH - 1), skip_group_check=True)
    # [1, *] scalar lane: mean, var, rs = exp(-0.5*ln(var+eps)), mu*rs
    sm = stats.tile([1, 2, NT], dt.float32, tag="sm", name=f"sm_{name}")
    nc.vector.tensor_copy(out=sm[:, 0, :], in_=st_sum[:, :])
    nc.vector.tensor_copy(out=sm[:, 1, :], in_=st_sq[:, :])
    nc.vector.tensor_scalar(out=sm[:, :, :], in0=sm[:, :, :],
                            scalar1=1.0 / D, scalar2=None, op0=Alu.mult)
    var = stats.tile([1, NT], dt.float32, tag="var", name=f"var_{name}")
    nc.vector.tensor_tensor(out=var[:, :], in0=sm[:, 0, :], in1=sm[:, 0, :],
                            op=Alu.mult)
    nc.vector.tensor_tensor(out=var[:, :], in0=sm[:, 1, :], in1=var[:, :],
                            op=Alu.subtract)
    # rsmu = [rs | mu*rs] on partition 0 (bf16: feeds the broadcast matmul)
    rsmu = stats.tile([1, 2, NT], dt.bfloat16, tag="rsmu", name=f"rsmu_{name}")
    nc.scalar.activation(out=var[:, :], in_=var[:, :], func=Act.Ln,
                         bias=eps_sb[:, :], scale=1.0)
    nc.scalar.activation(out=rsmu[:, 0, :], in_=var[:, :], func=Act.Exp,
                         bias=0.0, scale=-0.5)
    nc.vector.tensor_tensor(out=rsmu[:, 1, :], in0=sm[:, 0, :],
                            in1=rsmu[:, 0, :], op=Alu.mult)
    # broadcast rs and mu*rs across partitions via K=1 fp32 matmul
    bc = psum.tile([128, 2, NT], dt.float32, tag="sc", name=f"bc_{name}")
    nc.tensor.matmul(bc[:, 0, :], lhsT=ones_row_bf[:, :], rhs=rsmu[:, 0, :],
                     start=True, stop=True)
    nc.tensor.matmul(bc[:, 1, :], lhsT=ones_row_bf[:, :], rhs=rsmu[:, 1, :],
                     start=True, stop=True)
    # apply per chunk: x_out = (x*rs - mu*rs)*g + be, then bf16 cast.
    # Chunked ops let the next phase's matmuls start on early chunks via
    # Tile's subtile dependencies instead of waiting for the full tile.
    t = xres.tile([128, CH, NT], dt.float32, tag="x", name=f"t_{name}")
    x_out = xres.tile([128, CH, NT], dt.float32, tag="x", name=f"xo_{name}")
    x_out_bf = xbfp.tile([128, CH, NT], dt.bfloat16, tag="xb", bufs=1,
                         name=f"xob_{name}")
    for c in range(CH):
        nc.vector.tensor_tensor(out=t[:, c, :], in0=x_in[:, c, :],
                                in1=bc[:, 0, :], op=Alu.mult)
        nc.vector.tensor_tensor(out=t[:, c, :], in0=t[:, c, :],
                                in1=bc[:, 1, :], op=Alu.subtract)
        nc.vector.tensor_scalar(out=x_out[:, c, :], in0=t[:, c, :],
                                scalar1=g_sb[:, c:c + 1],
                                scalar2=be_sb[:, c:c + 1],
                                op0=Alu.mult, op1=Alu.add)
        nc.vector.tensor_copy(out=x_out_bf[:, c, :], in_=x_out[:, c, :])
    return x_out, x_out_bf


def build_program(layers=L):
    nc = bacc.Bacc("TRN2", target_bir_lowering=False, debug=False,
                   num_devices=NC)
    # ---------------- I/O ----------------
    tok_d = nc.dram_tensor("tokens_c", [NT], dt.int32, kind="ExternalInput")
    emb_d = nc.dram_tensor("emb", [V, D], dt.float32, kind="ExternalInput")
    pe_d = nc.dram_tensor("pe_fm", [D, NT], dt.float32, kind="ExternalInput")
    wq_d = nc.dram_tensor("Wq", [layers, D, D], dt.bfloat16, kind="ExternalInput")
    wk_d = nc.dram_tensor("Wk", [layers, D, D], dt.bfloat16, kind="ExternalInput")
    wv_d = nc.dram_tensor("Wv", [layers, D, D], dt.bfloat16, kind="ExternalInput")
    wo_d = nc.dram_tensor("Wo", [layers, D, D], dt.bfloat16, kind="ExternalInput")
    w1_d = nc.dram_tensor("W1", [layers, D, F], dt.bfloat16, kind="ExternalInput")
    w2_d = nc.dram_tensor("W2", [layers, F, D], dt.bfloat16, kind="ExternalInput")
    bq_d = nc.dram_tensor("bq", [layers, D], dt.float32, kind="ExternalInput")
    bk_d = nc.dram_tensor("bk", [layers, D], dt.float32, kind="ExternalInput")
    bv_d = nc.dram_tensor("bv", [layers, D], dt.float32, kind="ExternalInput")
    bo_d = nc.dram_tensor("bo", [layers, D], dt.float32, kind="ExternalInput")
    b1_d = nc.dram_tensor("b1", [layers, F], dt.float32, kind="ExternalInput")
    b2_d = nc.dram_tensor("b2", [layers, D], dt.float32, kind="ExternalInput")
    g1_d = nc.dram_tensor("g1", [layers, D], dt.float32, kind="ExternalInput")
    be1_d = nc.dram_tensor("be1", [layers, D], dt.float32, kind="ExternalInput")
    g2_d = nc.dram_tensor("g2", [layers, D], dt.float32, kind="ExternalInput")
    be2_d = nc.dram_tensor("be2", [layers, D], dt.float32, kind="ExternalInput")
    out_d = nc.dram_tensor("out_fm", [D, NT], dt.float32, kind="ExternalOutput")

    groups = [[0, 1, 2, 3], [4, 5, 6, 7]]

    with tile.TileContext(nc) as tc, ExitStack() as ctx:
        psum = ctx.enter_context(tc.tile_pool(name="psum", bufs=2, space="PSUM"))
        consts = ctx.enter_context(tc.tile_pool(name="consts", bufs=1))
        small = ctx.enter_context(tc.tile_pool(name="small", bufs=1))
        xres = ctx.enter_context(tc.tile_pool(name="xres", bufs=3))
        xbfp = ctx.enter_context(tc.tile_pool(name="xbf", bufs=2))
        qkvp = ctx.enter_context(tc.tile_pool(name="qkv", bufs=1))
        attnp = ctx.enter_context(tc.tile_pool(name="attn", bufs=4))
        kvsp = ctx.enter_context(tc.tile_pool(name="kvs", bufs=2))
        wpool = ctx.enter_context(tc.tile_pool(name="w", bufs=3))
        hpool = ctx.enter_context(tc.tile_pool(name="h", bufs=1))
        opool = ctx.enter_context(tc.tile_pool(name="o", bufs=1))
        stats = ctx.enter_context(tc.tile_pool(name="stats", bufs=1))
        dram = ctx.enter_context(tc.tile_pool(name="dram", bufs=2, space="DRAM"))

        # ---------------- constants ----------------
        ident = consts.tile([128, 128], dt.float32, name="ident")
        from concourse.masks import make_identity
        make_identity(nc, ident[:, :])
        ones_col_bf = consts.tile([128, 1], dt.bfloat16, name="ones_col")
        nc.vector.memset(ones_col_bf[:, :], 1.0)
        ones_row_bf = consts.tile([1, 128], dt.bfloat16, name="ones_row")
        nc.vector.memset(ones_row_bf[:, :], 1.0)
        ones_kl = consts.tile([128, TC], dt.bfloat16, name="ones_kl")
        nc.vector.memset(ones_kl[:, :], 1.0)
        eps_sb = consts.tile([1, 1], dt.float32, name="eps_sb")
        nc.vector.memset(eps_sb[:, :], EPS)

        # ---------------- per-chunk K|V gather buffers ----------------
        # own_c holds [K chunk c: 128 feats x 512 toks] ++ [V chunk c,
        # token-major with a ones column after each head's 64 feats:
        # 128 tok-part x 4 klocal x (65+65)]. The ones columns are written
        # once here; every layer's V DMA-out leaves them untouched, so the
        # AllGathered copy always carries the softmax-denominator column.
        KSZ = 128 * NT               # 65536
        VSZ = 128 * TC * 130         # 66560
        own_kv = []
        gth_kv = []
        for c in range(CH):
            own = dram.tile([KSZ + VSZ], dt.bfloat16, tag=f"own{c}", bufs=1,
                            name=f"own_kv{c}")
            ov = own[KSZ:].rearrange("(p kl a b) -> p kl a b", p=128, kl=TC,
                                     a=2, b=65)
            nc.sync.dma_start(out=ov[:, :, 0, 64:65], in_=ones_kl[:, :])
            nc.sync.dma_start(out=ov[:, :, 1, 64:65], in_=ones_kl[:, :])
            own_kv.append(own)
            gth_kv.append(dram.tile([4, KSZ + VSZ], dt.bfloat16,
                                    tag=f"gth{c}", bufs=1, name=f"gth_kv{c}"))

        # ---------------- embedding ----------------
        tok_sb = consts.tile([128, TC], dt.int32, name="tok_sb")
        nc.sync.dma_start(out=tok_sb[:, :],
                          in_=tok_d[:].rearrange("(c p) -> p c", p=128))
        pe_sb = xres.tile([128, CH, NT], dt.float32, tag="x", name="pe_sb")
        nc.sync.dma_start(out=pe_sb[:, :, :],
                          in_=pe_d[:, :].rearrange("(c p) q -> p c q", p=128))
        x = xres.tile([128, CH, NT], dt.float32, tag="x", name="x0")
        for tc4 in range(TC):
            gath = wpool.tile([128, D], dt.float32, tag="w", name=f"gath{tc4}")
            nc.gpsimd.indirect_dma_start(
                out=gath[:, :], out_offset=None, in_=emb_d[:, :],
                in_offset=IndirectOffsetOnAxis(ap=tok_sb[:, tc4:tc4 + 1], axis=0))
            for fc in range(CH):
                tp = psum.tile([128, NT], dt.float32, tag="acc",
                               name=f"tp{tc4}_{fc}")
                nc.tensor.transpose(tp[:, 0:128],
                                    gath[:, fc * 128:(fc + 1) * 128],
                                    ident[:, :])
                nc.vector.tensor_scalar(
                    out=x[:, fc, tc4 * 128:(tc4 + 1) * 128], in0=tp[:, 0:128],
                    scalar1=float(SQRT_D), scalar2=None, op0=Alu.mult)
        nc.vector.tensor_tensor(out=x[:, :, :], in0=x[:, :, :],
                                in1=pe_sb[:, :, :], op=Alu.add)
        x_bf = xbfp.tile([128, CH, NT], dt.bfloat16, tag="xb", bufs=1,
                         name="x0_bf")
        nc.vector.tensor_copy(out=x_bf[:, :, :], in_=x[:, :, :])

        # ---------------- layers ----------------
        for l in range(layers):
            wq_r = wq_d[l].rearrange("(kc p) f -> p kc f", p=128)
            wk_r = wk_d[l].rearrange("(kc p) f -> p kc f", p=128)
            wv_r = wv_d[l].rearrange("(kc p) f -> p kc f", p=128)
            wo_r = wo_d[l].rearrange("(kc p) f -> p kc f", p=128)
            w1_r = w1_d[l].rearrange("(kc p) f -> p kc f", p=128)
            w2_r = w2_d[l].rearrange("(kc p) f -> p kc f", p=128)

            bq_sb = small.tile([128, CH], dt.float32, tag="bq", name=f"bq{l}")
            nc.sync.dma_start(out=bq_sb[:, :],
                              in_=bq_d[l].rearrange("(c p) -> p c", p=128))
            bk_sb = small.tile([128, CH], dt.float32, tag="bk", name=f"bk{l}")
            nc.sync.dma_start(out=bk_sb[:, :],
                              in_=bk_d[l].rearrange("(c p) -> p c", p=128))
            bo_sb = small.tile([128, CH], dt.float32, tag="bo", name=f"bo{l}")
            nc.sync.dma_start(out=bo_sb[:, :],
                              in_=bo_d[l].rearrange("(c p) -> p c", p=128))
            b2_sb = small.tile([128, CH], dt.float32, tag="b2", name=f"b2{l}")
            nc.sync.dma_start(out=b2_sb[:, :],
                              in_=b2_d[l].rearrange("(c p) -> p c", p=128))
            b1_sb = small.tile([128, FCH], dt.float32, tag="b1", name=f"b1{l}")
            nc.sync.dma_start(out=b1_sb[:, :],
                              in_=b1_d[l].rearrange("(c p) -> p c", p=128))
            g1_sb = small.tile([128, CH], dt.float32, tag="g1", name=f"g1{l}")
            nc.sync.dma_start(out=g1_sb[:, :],
                              in_=g1_d[l].rearrange("(c p) -> p c", p=128))
            be1_sb = small.tile([128, CH], dt.float32, tag="be1", name=f"be1{l}")
            nc.sync.dma_start(out=be1_sb[:, :],
                              in_=be1_d[l].rearrange("(c p) -> p c", p=128))
            g2_sb = small.tile([128, CH], dt.float32, tag="g2", name=f"g2{l}")
            nc.sync.dma_start(out=g2_sb[:, :],
                              in_=g2_d[l].rearrange("(c p) -> p c", p=128))
            be2_sb = small.tile([128, CH], dt.float32, tag="be2", name=f"be2{l}")
            nc.sync.dma_start(out=be2_sb[:, :],
                              in_=be2_d[l].rearrange("(c p) -> p c", p=128))
            bv_l = bv_d[l]
            bv_bc = small.tile([128, D], dt.float32, tag="bv", name=f"bv{l}")
            nc.gpsimd.dma_start(
                out=bv_bc[:, :],
                in_=bass.AP(tensor=bv_l.tensor, offset=bv_l.offset,
                            ap=[[0, 128]] + list(bv_l.ap)))

            # ---- K/V projections; a combined [K|V] AllGather is kicked per
            # feature chunk (= head pair) so attention chunk c can start as
            # soon as its own small gather lands, and the 8 gathers pipeline
            # under the whole attention phase. ----
            # K is feature-major [1024, NT]; V is token-major [NT, 1024].
            k_bf = qkvp.tile([128, CH, NT], dt.bfloat16, tag="k", name=f"k{l}")
            v_bf = qkvp.tile([128, TC, D], dt.bfloat16, tag="v", name=f"v{l}")

            def k_chunk(m):
                wm = wpool.tile([128, CH, 128], dt.bfloat16, tag="w",
                                name=f"wk{l}_{m}")
                nc.sync.dma_start(out=wm[:, :, :],
                                  in_=wk_r[:, :, m * 128:(m + 1) * 128])
                ps = psum.tile([128, NT], dt.float32, tag="acc",
                               name=f"psk{l}_{m}")
                for kc in range(CH):
                    nc.tensor.matmul(ps[:, :], lhsT=wm[:, kc, :],
                                     rhs=x_bf[:, kc, :],
                                     start=(kc == 0), stop=(kc == CH - 1))
                nc.vector.tensor_scalar(out=k_bf[:, m, :], in0=ps[:, :],
                                        scalar1=bk_sb[:, m:m + 1],
                                        scalar2=None, op0=Alu.add)

            def v_half(hf):
                wvn = wpool.tile([128, CH, 512], dt.bfloat16, tag="w",
                                 name=f"wv{l}_{hf}")
                nc.sync.dma_start(out=wvn[:, :, :],
                                  in_=wv_r[:, :, hf * 512:(hf + 1) * 512])
                for tc4 in range(TC):
                    ps = psum.tile([128, 512], dt.float32, tag="acc",
                                   name=f"psv{l}_{hf}_{tc4}")
                    for kc in range(CH):
                        nc.tensor.matmul(
                            ps[:, :],
                            lhsT=x_bf[:, kc, tc4 * 128:(tc4 + 1) * 128],
                            rhs=wvn[:, kc, :],
                            start=(kc == 0), stop=(kc == CH - 1))
                    nc.vector.tensor_tensor(
                        out=v_bf[:, tc4, hf * 512:(hf + 1) * 512], in0=ps[:, :],
                        in1=bv_bc[:, hf * 512:(hf + 1) * 512], op=Alu.add)

            v_bf_hsplit = v_bf[:, :, :].rearrange("p t (c a b) -> p t c a b",
                                                  a=2, b=64)

            def kv_gather(c):
                own = own_kv[c]
                nc.sync.dma_start(
                    out=own[0:KSZ].rearrange("(p q) -> p q", p=128),
                    in_=k_bf[:, c, :])
                ov = own[KSZ:].rearrange("(p kl a b) -> p kl a b", p=128,
                                         kl=TC, a=2, b=65)
                for i in range(2):
                    nc.gpsimd.dma_start(out=ov[:, :, i, 0:64],
                                        in_=v_bf_hsplit[:, :, c, i, :])
                nc.gpsimd.collective_compute(
                    "AllGather", Alu.bypass, replica_groups=groups,
                    ins=[own[:].opt()], outs=[gth_kv[c][:, :].opt()])

            for m in range(4):
                k_chunk(m)
            v_half(0)
            for c in range(4):
                kv_gather(c)
            for m in range(4, CH):
                k_chunk(m)
            v_half(1)
            for c in range(4, CH):
                kv_gather(c)

            # ---- Q projection (feature-major; head pair per chunk) ----
            q_bf = qkvp.tile([128, CH, NT], dt.bfloat16, tag="q", name=f"q{l}")
            for m in range(CH):
                wm = wpool.tile([128, CH, 128], dt.bfloat16, tag="w",
                                name=f"wq{l}_{m}")
                nc.sync.dma_start(out=wm[:, :, :],
                                  in_=wq_r[:, :, m * 128:(m + 1) * 128])
                ps = psum.tile([128, NT], dt.float32, tag="acc",
                               name=f"psq{l}_{m}")
                for kc in range(CH):
                    nc.tensor.matmul(ps[:, :], lhsT=wm[:, kc, :],
                                     rhs=x_bf[:, kc, :],
                                     start=(kc == 0), stop=(kc == CH - 1))
                nc.vector.tensor_scalar(out=q_bf[:, m, :], in0=ps[:, :],
                                        scalar1=bq_sb[:, m:m + 1],
                                        scalar2=None, op0=Alu.add)

            # ---- attention: one head PAIR per feature chunk; the two heads'
            # scores matmuls row-pack the PE array (K=64 each). The kc loop
            # runs scores+exp LAG steps ahead of the AV accumulation so the
            # Scalar exp stream (the phase's rate limiter) never waits on the
            # PE and vice versa. Wo runs as two half-passes interleaved into
            # the stream as PE filler, accumulating into x1a / x1. ----
            LAG = 4
            o_nbf = opool.tile([128, CH, NT], dt.bfloat16, tag="onb",
                               name=f"onb{l}")
            x1a = xres.tile([128, CH, NT], dt.bfloat16, tag="x1a", bufs=1,
                            name=f"x1a_{l}")
            x1 = xres.tile([128, CH, NT], dt.float32, tag="x", name=f"x1_{l}")
            filler = []

            def emit_filler(n):
                for _ in range(min(n, len(filler))):
                    filler.pop(0)()

            def wo_pass(half, wo_sb):
                # one output chunk m per filler slot: 4 accumulation matmuls
                # + one DVE op folding into x1a (pass A) or x1 (pass B)
                def mk(m):
                    def th():
                        ps = psum.tile([128, NT], dt.float32, tag="acc",
                                       name=f"pso{l}_{half}_{m}")
                        for kc in range(4):
                            nc.tensor.matmul(
                                ps[:, :], lhsT=wo_sb[:, kc, m * 128:(m + 1) * 128],
                                rhs=o_nbf[:, 4 * half + kc, :],
                                start=(kc == 0), stop=(kc == 3))
                        if half == 0:
                            nc.vector.tensor_scalar(out=x1a[:, m, :],
                                                    in0=ps[:, :],
                                                    scalar1=bo_sb[:, m:m + 1],
                                                    scalar2=None, op0=Alu.add)
                        else:
                            nc.vector.tensor_tensor(out=x1[:, m, :],
                                                    in0=ps[:, :],
                                                    in1=x1a[:, m, :],
                                                    op=Alu.add)
                            nc.vector.tensor_tensor(out=x1[:, m, :],
                                                    in0=x1[:, m, :],
                                                    in1=x[:, m, :], op=Alu.add)
                    return th
                for m in range(CH):
                    filler.append(mk(m))

            wo_sbs = []
            for c in range(CH):
                gv = gth_kv[c]
                k_sb = kvsp.tile([128, 4, NT], dt.bfloat16, tag="k",
                                 name=f"ksb{l}_{c}")
                nc.sync.dma_start(
                    out=k_sb[:, :, :],
                    in_=gv[:, 0:KSZ].rearrange("g (p q) -> p g q", p=128))
                v_sb = kvsp.tile([128, 4, TC, 130], dt.bfloat16, tag="v",
                                 name=f"vsb{l}_{c}")
                nc.gpsimd.dma_start(
                    out=v_sb[:, :, :, :],
                    in_=gv[:, KSZ:].rearrange("g (p kl f) -> p g kl f",
                                              p=128, f=130))
                avs = [psum.tile([128, NT], dt.float32, tag="av",
                                 name=f"av{l}_{2 * c + i}") for i in range(2)]
                ats = {}
                for kc in range(KCH + LAG):
                    if kc >= LAG:
                        j = kc - LAG
                        at = ats.pop(j)
                        for i in range(2):
                            nc.tensor.matmul(
                                avs[i][0:65, :],
                                lhsT=v_sb[:, j // 4, j % 4,
                                          65 * i:65 * i + 65],
                                rhs=at[:, i, :], start=(j == 0),
                                stop=(j == KCH - 1),
                                skip_group_check=True)
                        emit_filler(1)
                    if kc < KCH:
                        sc = psum.tile([128, 2, NT], dt.float32, tag="sc",
                                       name=f"sc{l}_{c}_{kc}")
                        for i in range(2):
                            nc.tensor.matmul(
                                sc[:, i, :],
                                lhsT=k_sb[64 * i:64 * i + 64, kc // 4,
                                          (kc % 4) * 128:(kc % 4 + 1) * 128],
                                rhs=q_bf[64 * i:64 * i + 64, c, :],
                                start=True, stop=True,
                                tile_position=(64 * i, 0))
                        at = attnp.tile([128, 2, NT], dt.bfloat16, tag="at",
                                        bufs=6, name=f"at{l}_{c}_{kc}")
                        nc.scalar.activation(out=at[:, :, :],
                                             in_=sc[:, :, :],
                                             func=Act.Exp, bias=0.0,
                                             scale=0.125)
                        ats[kc] = at
                # normalize by the softmax denominator (row 64 of av); this
                # executes under the next chunk's scores/exp stream.
                for i in range(2):
                    h = 2 * c + i
                    av = avs[i]
                    o_h = attnp.tile([64, NT], dt.bfloat16, tag="oh", bufs=2,
                                     name=f"oh{l}_{h}")
                    nc.vector.tensor_copy(out=o_h[:, :], in_=av[0:64, :])
                    z_h = stats.tile([1, NT], dt.float32, tag="zh",
                                     name=f"zh{l}_{h}")
                    nc.vector.tensor_copy(out=z_h[:, :], in_=av[64:65, :])
                    zi_f = stats.tile([1, NT], dt.float32, tag="zif",
                                      name=f"zif{l}_{h}")
                    nc.vector.reciprocal_approx_fast(out=zi_f[:, :],
                                                     in_=z_h[:, :])
                    zi_h = stats.tile([1, NT], dt.bfloat16, tag="zi",
                                      name=f"zi{l}_{h}")
                    with nc.allow_low_precision(reason="1/Z feeds a bf16 "
                                                "broadcast matmul anyway"):
                        nc.vector.tensor_copy(out=zi_h[:, :], in_=zi_f[:, :])
                    bz = psum.tile([64, NT], dt.float32, tag="acc",
                                   name=f"bz{l}_{h}")
                    nc.tensor.matmul(bz[:, :], lhsT=ones_row_bf[0:1, 0:64],
                                     rhs=zi_h[:, :], start=True, stop=True)
                    nc.vector.tensor_tensor(out=o_nbf[64 * i:64 * i + 64, c, :],
                                            in0=o_h[:, :], in1=bz[:, :],
                                            op=Alu.mult)
                # prefetch the Wo weight half needed one chunk later, then
                # queue its 8 output-chunk thunks as PE filler
                if c in (2, 6):
                    half = (c - 2) // 4
                    wo_sb = wpool.tile([128, 4, D], dt.bfloat16, tag="wo",
                                       bufs=1, name=f"wo{l}_{half}")
                    nc.sync.dma_start(out=wo_sb[:, :, :],
                                      in_=wo_r[:, 4 * half:4 * half + 4, :])
                    wo_sbs.append(wo_sb)
                if c in (3, 7):
                    wo_pass((c - 3) // 4, wo_sbs[(c - 3) // 4])
            emit_filler(len(filler))

            x1n, x1n_bf = _ln(ctx, nc, psum, xres, xbfp, stats, x1, g1_sb,
                              be1_sb, ones_col_bf, ones_row_bf, eps_sb,
                              f"l{l}a")

            # ---- FFN ----
            x2 = xres.tile([128, CH, NT], dt.float32, tag="x", name=f"x2_{l}")
            for half in range(2):
                h_bf = hpool.tile([128, FCH // 2, NT], dt.bfloat16, tag="h",
                                  name=f"h{l}_{half}")
                for mi in range(FCH // 2):
                    m = half * (FCH // 2) + mi
                    w1m = wpool.tile([128, CH, 128], dt.bfloat16, tag="w",
                                     name=f"w1_{l}_{m}")
                    nc.sync.dma_start(out=w1m[:, :, :],
                                      in_=w1_r[:, :, m * 128:(m + 1) * 128])
                    ps = psum.tile([128, NT], dt.float32, tag="acc",
                                   name=f"ps1{l}_{m}")
                    for kc in range(CH):
                        nc.tensor.matmul(ps[:, :], lhsT=w1m[:, kc, :],
                                         rhs=x1n_bf[:, kc, :],
                                         start=(kc == 0), stop=(kc == CH - 1))
                    nc.vector.tensor_scalar(out=h_bf[:, mi, :], in0=ps[:, :],
                                            scalar1=b1_sb[:, m:m + 1],
                                            scalar2=0.0, op0=Alu.add,
                                            op1=Alu.max)
                for m in range(CH):
                    w2m = wpool.tile([128, FCH // 2, 128], dt.bfloat16, tag="w",
                                     name=f"w2_{l}_{half}_{m}")
                    nc.sync.dma_start(
                        out=w2m[:, :, :],
                        in_=w2_r[:, half * (FCH // 2):(half + 1) * (FCH // 2),
                                 m * 128:(m + 1) * 128])
                    ps = psum.tile([128, NT], dt.float32, tag="acc",
                                   name=f"ps2{l}_{half}_{m}")
                    for kc in range(FCH // 2):
                        nc.tensor.matmul(ps[:, :], lhsT=w2m[:, kc, :],
                                         rhs=h_bf[:, kc, :],
                                         start=(kc == 0),
                                         stop=(kc == FCH // 2 - 1))
                    if half == 0:
                        nc.vector.tensor_scalar(out=x2[:, m, :], in0=ps[:, :],
                                                scalar1=b2_sb[:, m:m + 1],
                                                scalar2=None, op0=Alu.add)
                    else:
                        nc.vector.tensor_tensor(out=x2[:, m, :],
                                                in0=x2[:, m, :], in1=ps[:, :],
                                                op=Alu.add)
                        nc.vector.tensor_tensor(out=x2[:, m, :],
                                                in0=x2[:, m, :],
                                                in1=x1n[:, m, :], op=Alu.add)

            x, x_bf = _ln(ctx, nc, psum, xres, xbfp, stats, x2, g2_sb, be2_sb,
                          ones_col_bf, ones_row_bf, eps_sb, f"l{l}b")

        # ---------------- output ----------------
        nc.sync.dma_start(
            out=out_d[:, :].rearrange("(c p) q -> p c q", p=128),
            in_=x[:, :, :])

    nc.compile()
    return nc


_PROG = {}


def _get_prog(layers=L):
    if layers not in _PROG:
        _PROG[layers] = build_program(layers)
    return _PROG[layers]


def _host_inputs(inputs, layers=L):
    """Build the 8 per-core input maps from the full-model inputs."""
    bf16 = ml_dtypes.bfloat16
    f32 = np.float32
    tokens = np.asarray(inputs["tokens"])
    # positional encoding (constant)
    pos = np.arange(S)[:, None].astype(f32)
    freq = np.exp(np.arange(0, D, 2).astype(f32) * -(math.log(10000.0) / D))
    pe = np.zeros((S, D), dtype=f32)
    pe[:, 0::2] = np.sin(pos * freq)
    pe[:, 1::2] = np.cos(pos * freq)
    shared = {
        "emb": np.ascontiguousarray(np.asarray(inputs["emb"], dtype=f32)),
    }
    for nm in ("Wq", "Wk", "Wv", "Wo", "W1", "W2"):
        shared[nm] = np.ascontiguousarray(
            np.asarray(inputs[nm])[:layers].astype(bf16))
    for nm in ("bq", "bk", "bv", "bo", "b1", "b2", "g1", "be1", "g2", "be2"):
        shared[nm] = np.ascontiguousarray(
            np.asarray(inputs[nm])[:layers].astype(f32))

    in_maps = []
    for core in range(NC):
        b, lc = core // 4, core % 4
        m = dict(shared)
        m["tokens_c"] = np.ascontiguousarray(tokens[b, lc * NT:(lc + 1) * NT])
        m["pe_fm"] = np.ascontiguousarray(pe[lc * NT:(lc + 1) * NT, :].T)
        in_maps.append(m)
    return in_maps


def run(inputs, layers=L, trace=False):
    nc = _get_prog(layers)
    in_maps = _host_inputs(inputs, layers)
    last_err = None
    for attempt in range(3):
        try:
            res = run_bass_kernel_spmd(nc, in_maps, list(range(NC)),
                                       trace=trace)
            break
        except Exception as e:  # transient NRT device faults observed
            last_err = e
            if attempt == 2:
                raise
            import time
            time.sleep(5)
    out = np.zeros((B, S, D), dtype=np.float32)
    for core in range(NC):
        b, lc = core // 4, core % 4
        out[b, lc * NT:(lc + 1) * NT, :] = res.results[core]["out_fm"].T
    return out, res


def kernel(**inputs):
    out, _ = run(inputs)
    return out



# revision 15
# speedup vs baseline: 1.1462x; 1.0366x over previous
"""Self-contained Trainium2 Bass kernel for a 6-layer post-LN transformer
encoder (B=2, S=2048, D=1024, H=16, F=4096, V=32000).

Sharding: sequence-parallel across 8 NeuronCores. Core c handles batch
b = c // 4, sequence slice [lc*512, (lc+1)*512) with lc = c % 4. Per layer,
one AllGather (replica groups [[0..3],[4..7]]) shares bf16 K/V across the
4 cores of each batch; all other compute is local to the core.

On-chip layout: residual stream is feature-major [1024 feats -> 8 chunks of
128 partitions, 512 tokens free] in f32. Matmul inputs are cast to bf16;
PSUM accumulation is f32. Attention scores are computed pre-transposed
(K @ Q^T per head) so softmax exp needs no transposes; the softmax
denominator comes from a ones-column appended to V in the AV matmul.
LayerNorm stats use ones-vector matmuls over the partition dim; rsqrt is
computed as exp(-0.5*ln(var+eps)) to stay in one ACT table set.
"""

import math
from contextlib import ExitStack

import ml_dtypes
import numpy as np

import concourse.bass as bass
import concourse.tile as tile
from concourse import bacc, mybir
from concourse.bass import IndirectOffsetOnAxis
from concourse.bass_utils import run_bass_kernel_spmd

dt = mybir.dt
Alu = mybir.AluOpType
Act = mybir.ActivationFunctionType

V, L, D, S, F, H, B = 32000, 6, 1024, 2048, 4096, 16, 2
DK = D // H          # 64
NC = 8               # cores
NT = (B * S) // NC   # 512 tokens per core
CH = D // 128        # 8 feature chunks
FCH = F // 128       # 32 ffn chunks
TC = NT // 128       # 4 token chunks per core
KCH = S // 128       # 16 key chunks per batch
EPS = 1e-5
SQRT_D = math.sqrt(D)
KV_HALF = D * NT     # flat elems of K (or V) contribution per core


def _ln(ctx, nc, psum, xres, xbfp, stats, x_in, g_sb, be_sb, ones_col_bf,
        ones_row_bf, eps_sb, name):
    """LayerNorm over features (partition dim) of feature-major x_in
    [128, CH, 512] f32. Returns (x_out f32 tile, x_out bf16 tile)."""
    # bf16 copies for stats matmuls, in a rotating 2-chunk scratch;
    # column sums via ones-matmuls accumulated over feature chunks
    x_b = xbfp.tile([128, 2, NT], dt.bfloat16, tag="sq", bufs=1,
                    name=f"xb_{name}")
    x_sq = xbfp.tile([128, 2, NT], dt.bfloat16, tag="sq2", bufs=1,
                     name=f"xsq_{name}")
    st_sum = psum.tile([1, NT], dt.float32, tag="acc", name=f"sts_{name}")
    st_sq = psum.tile([1, NT], dt.float32, tag="acc", name=f"stq_{name}")
    for kc in range(CH):
        s = kc % 2
        nc.vector.tensor_copy(out=x_b[:, s, :], in_=x_in[:, kc, :])
        nc.vector.tensor_tensor(out=x_sq[:, s, :], in0=x_b[:, s, :],
                                in1=x_b[:, s, :], op=Alu.mult)
        nc.tensor.matmul(st_sum[:, :], lhsT=ones_col_bf[:, :],
                         rhs=x_b[:, s, :], start=(kc == 0),
                         stop=(kc == CH - 1), skip_group_check=True)
        nc.tensor.matmul(st_sq[:, :], lhsT=ones_col_bf[:, :],
                         rhs=x_sq[:, s, :], start=(kc == 0),
                         stop=(kc == CH - 1), skip_group_check=True)
    # [1, *] scalar lane: mean, var, rs = exp(-0.5*ln(var+eps)), mu*rs
    sm = stats.tile([1, 2, NT], dt.float32, tag="sm", name=f"sm_{name}")
    nc.vector.tensor_copy(out=sm[:, 0, :], in_=st_sum[:, :])
    nc.vector.tensor_copy(out=sm[:, 1, :], in_=st_sq[:, :])
    nc.vector.tensor_scalar(out=sm[:, :, :], in0=sm[:, :, :],
                            scalar1=1.0 / D, scalar2=None, op0=Alu.mult)
    var = stats.tile([1, NT], dt.float32, tag="var", name=f"var_{name}")
    nc.vector.tensor_tensor(out=var[:, :], in0=sm[:, 0, :], in1=sm[:, 0, :],
                            op=Alu.mult)
    nc.vector.tensor_tensor(out=var[:, :], in0=sm[:, 1, :], in1=var[:, :],
                            op=Alu.subtract)
    # rsmu = [rs | mu*rs] on partition 0 (bf16: feeds the broadcast matmul)
    rsmu = stats.tile([1, 2, NT], dt.bfloat16, tag="rsmu", name=f"rsmu_{name}")
    nc.scalar.activation(out=var[:, :], in_=var[:, :], func=Act.Ln,
                         bias=eps_sb[:, :], scale=1.0)
    nc.scalar.activation(out=rsmu[:, 0, :], in_=var[:, :], func=Act.Exp,
                         bias=0.0, scale=-0.5)
    nc.vector.tensor_tensor(out=rsmu[:, 1, :], in0=sm[:, 0, :],
                            in1=rsmu[:, 0, :], op=Alu.mult)
    # broadcast rs and mu*rs across partitions via K=1 fp32 matmul
    bc = psum.tile([128, 2, NT], dt.float32, tag="sc", name=f"bc_{name}")
    nc.tensor.matmul(bc[:, 0, :], lhsT=ones_row_bf[:, :], rhs=rsmu[:, 0, :],
                     start=True, stop=True)
    nc.tensor.matmul(bc[:, 1, :], lhsT=ones_row_bf[:, :], rhs=rsmu[:, 1, :],
                     start=True, stop=True)
    # apply per chunk: x_out = (x*rs - mu*rs)*g + be, then bf16 cast.
    # Chunked ops let the next phase's matmuls start on early chunks via
    # Tile's subtile dependencies instead of waiting for the full tile.
    t = xres.tile([128, CH, NT], dt.float32, tag="x", name=f"t_{name}")
    x_out = xres.tile([128, CH, NT], dt.float32, tag="x", name=f"xo_{name}")
    x_out_bf = xbfp.tile([128, CH, NT], dt.bfloat16, tag="xb", bufs=1,
                         name=f"xob_{name}")
    for c in range(CH):
        nc.vector.tensor_tensor(out=t[:, c, :], in0=x_in[:, c, :],
                                in1=bc[:, 0, :], op=Alu.mult)
        nc.vector.tensor_tensor(out=t[:, c, :], in0=t[:, c, :],
                                in1=bc[:, 1, :], op=Alu.subtract)
        nc.vector.tensor_scalar(out=x_out[:, c, :], in0=t[:, c, :],
                                scalar1=g_sb[:, c:c + 1],
                                scalar2=be_sb[:, c:c + 1],
                                op0=Alu.mult, op1=Alu.add)
        nc.vector.tensor_copy(out=x_out_bf[:, c, :], in_=x_out[:, c, :])
    return x_out, x_out_bf


def build_program(layers=L):
    nc = bacc.Bacc("TRN2", target_bir_lowering=False, debug=False,
                   num_devices=NC)
    # ---------------- I/O ----------------
    tok_d = nc.dram_tensor("tokens_c", [NT], dt.int32, kind="ExternalInput")
    emb_d = nc.dram_tensor("emb", [V, D], dt.float32, kind="ExternalInput")
    pe_d = nc.dram_tensor("pe_fm", [D, NT], dt.float32, kind="ExternalInput")
    wq_d = nc.dram_tensor("Wq", [layers, D, D], dt.bfloat16, kind="ExternalInput")
    wk_d = nc.dram_tensor("Wk", [layers, D, D], dt.bfloat16, kind="ExternalInput")
    wv_d = nc.dram_tensor("Wv", [layers, D, D], dt.bfloat16, kind="ExternalInput")
    wo_d = nc.dram_tensor("Wo", [layers, D, D], dt.bfloat16, kind="ExternalInput")
    w1_d = nc.dram_tensor("W1", [layers, D, F], dt.bfloat16, kind="ExternalInput")
    w2_d = nc.dram_tensor("W2", [layers, F, D], dt.bfloat16, kind="ExternalInput")
    bq_d = nc.dram_tensor("bq", [layers, D], dt.float32, kind="ExternalInput")
    bk_d = nc.dram_tensor("bk", [layers, D], dt.float32, kind="ExternalInput")
    bv_d = nc.dram_tensor("bv", [layers, D], dt.float32, kind="ExternalInput")
    bo_d = nc.dram_tensor("bo", [layers, D], dt.float32, kind="ExternalInput")
    b1_d = nc.dram_tensor("b1", [layers, F], dt.float32, kind="ExternalInput")
    b2_d = nc.dram_tensor("b2", [layers, D], dt.float32, kind="ExternalInput")
    g1_d = nc.dram_tensor("g1", [layers, D], dt.float32, kind="ExternalInput")
    be1_d = nc.dram_tensor("be1", [layers, D], dt.float32, kind="ExternalInput")
    g2_d = nc.dram_tensor("g2", [layers, D], dt.float32, kind="ExternalInput")
    be2_d = nc.dram_tensor("be2", [layers, D], dt.float32, kind="ExternalInput")
    out_d = nc.dram_tensor("out_fm", [D, NT], dt.float32, kind="ExternalOutput")

    groups = [[0, 1, 2, 3], [4, 5, 6, 7]]

    with tile.TileContext(nc) as tc, ExitStack() as ctx:
        psum = ctx.enter_context(tc.tile_pool(name="psum", bufs=2, space="PSUM"))
        consts = ctx.enter_context(tc.tile_pool(name="consts", bufs=1))
        small = ctx.enter_context(tc.tile_pool(name="small", bufs=1))
        xres = ctx.enter_context(tc.tile_pool(name="xres", bufs=3))
        xbfp = ctx.enter_context(tc.tile_pool(name="xbf", bufs=2))
        qkvp = ctx.enter_context(tc.tile_pool(name="qkv", bufs=1))
        attnp = ctx.enter_context(tc.tile_pool(name="attn", bufs=4))
        kvsp = ctx.enter_context(tc.tile_pool(name="kvs", bufs=2))
        wpool = ctx.enter_context(tc.tile_pool(name="w", bufs=3))
        hpool = ctx.enter_context(tc.tile_pool(name="h", bufs=1))
        opool = ctx.enter_context(tc.tile_pool(name="o", bufs=1))
        stats = ctx.enter_context(tc.tile_pool(name="stats", bufs=1))
        dram = ctx.enter_context(tc.tile_pool(name="dram", bufs=2, space="DRAM"))

        # ---------------- constants ----------------
        ident = consts.tile([128, 128], dt.float32, name="ident")
        from concourse.masks import make_identity
        make_identity(nc, ident[:, :])
        ones_col_bf = consts.tile([128, 1], dt.bfloat16, name="ones_col")
        nc.vector.memset(ones_col_bf[:, :], 1.0)
        ones_row_bf = consts.tile([1, 128], dt.bfloat16, name="ones_row")
        nc.vector.memset(ones_row_bf[:, :], 1.0)
        ones_kl = consts.tile([128, TC], dt.bfloat16, name="ones_kl")
        nc.vector.memset(ones_kl[:, :], 1.0)
        eps_sb = consts.tile([1, 1], dt.float32, name="eps_sb")
        nc.vector.memset(eps_sb[:, :], EPS)

        # ---------------- per-chunk K|V gather buffers ----------------
        # own_c holds [K chunk c: 128 feats x 512 toks] ++ [V chunk c,
        # token-major with a ones column after each head's 64 feats:
        # 128 tok-part x 4 klocal x (65+65)]. The ones columns are written
        # once here; every layer's V DMA-out leaves them untouched, so the
        # AllGathered copy always carries the softmax-denominator column.
        KSZ = 128 * NT               # 65536
        VSZ = 128 * TC * 130         # 66560
        own_kv = []
        gth_kv = []
        for c in range(CH):
            own = dram.tile([KSZ + VSZ], dt.bfloat16, tag=f"own{c}", bufs=1,
                            name=f"own_kv{c}")
            ov = own[KSZ:].rearrange("(p kl a b) -> p kl a b", p=128, kl=TC,
                                     a=2, b=65)
            nc.sync.dma_start(out=ov[:, :, 0, 64:65], in_=ones_kl[:, :])
            nc.sync.dma_start(out=ov[:, :, 1, 64:65], in_=ones_kl[:, :])
            own_kv.append(own)
            gth_kv.append(dram.tile([4, KSZ + VSZ], dt.bfloat16,
                                    tag=f"gth{c}", bufs=1, name=f"gth_kv{c}"))

        # ---------------- bias/gain preload (all layers at once) ----------
        # The per-layer "(c p) -> p c" loads are 4-byte-per-partition
        # scatter DMAs costing multiple us each on the sync queue; one
        # full-depth DMA per tensor at startup removes ~9 slow DMAs/layer.
        def _preload(t_d, nch, nm):
            tl = small.tile([128, layers, nch], dt.float32, tag=nm, name=nm)
            nc.sync.dma_start(out=tl[:, :, :],
                              in_=t_d[:, :].rearrange("l (c p) -> p l c",
                                                      p=128))
            return tl
        bq_all = _preload(bq_d, CH, "bqa")
        bk_all = _preload(bk_d, CH, "bka")
        bo_all = _preload(bo_d, CH, "boa")
        b2_all = _preload(b2_d, CH, "b2a")
        b1_all = _preload(b1_d, FCH, "b1a")
        g1_all = _preload(g1_d, CH, "g1a")
        be1_all = _preload(be1_d, CH, "be1a")
        g2_all = _preload(g2_d, CH, "g2a")
        be2_all = _preload(be2_d, CH, "be2a")

        # ---------------- embedding ----------------
        tok_sb = consts.tile([128, TC], dt.int32, name="tok_sb")
        nc.sync.dma_start(out=tok_sb[:, :],
                          in_=tok_d[:].rearrange("(c p) -> p c", p=128))
        pe_sb = xres.tile([128, CH, NT], dt.float32, tag="x", name="pe_sb")
        nc.sync.dma_start(out=pe_sb[:, :, :],
                          in_=pe_d[:, :].rearrange("(c p) q -> p c q", p=128))
        x = xres.tile([128, CH, NT], dt.float32, tag="x", name="x0")
        for tc4 in range(TC):
            gath = wpool.tile([128, D], dt.float32, tag="w", name=f"gath{tc4}")
            nc.gpsimd.indirect_dma_start(
                out=gath[:, :], out_offset=None, in_=emb_d[:, :],
                in_offset=IndirectOffsetOnAxis(ap=tok_sb[:, tc4:tc4 + 1], axis=0))
            for fc in range(CH):
                tp = psum.tile([128, NT], dt.float32, tag="acc",
                               name=f"tp{tc4}_{fc}")
                nc.tensor.transpose(tp[:, 0:128],
                                    gath[:, fc * 128:(fc + 1) * 128],
                                    ident[:, :])
                nc.vector.tensor_scalar(
                    out=x[:, fc, tc4 * 128:(tc4 + 1) * 128], in0=tp[:, 0:128],
                    scalar1=float(SQRT_D), scalar2=None, op0=Alu.mult)
        nc.vector.tensor_tensor(out=x[:, :, :], in0=x[:, :, :],
                                in1=pe_sb[:, :, :], op=Alu.add)
        x_bf = xbfp.tile([128, CH, NT], dt.bfloat16, tag="xb", bufs=1,
                         name="x0_bf")
        nc.vector.tensor_copy(out=x_bf[:, :, :], in_=x[:, :, :])

        # ---------------- layers ----------------
        for l in range(layers):
            wq_r = wq_d[l].rearrange("(kc p) f -> p kc f", p=128)
            wk_r = wk_d[l].rearrange("(kc p) f -> p kc f", p=128)
            wv_r = wv_d[l].rearrange("(kc p) f -> p kc f", p=128)
            wo_r = wo_d[l].rearrange("(kc p) f -> p kc f", p=128)
            w1_r = w1_d[l].rearrange("(kc p) f -> p kc f", p=128)
            w2_r = w2_d[l].rearrange("(kc p) f -> p kc f", p=128)

            bq_sb = bq_all[:, l, :]
            bk_sb = bk_all[:, l, :]
            bo_sb = bo_all[:, l, :]
            b2_sb = b2_all[:, l, :]
            b1_sb = b1_all[:, l, :]
            g1_sb = g1_all[:, l, :]
            be1_sb = be1_all[:, l, :]
            g2_sb = g2_all[:, l, :]
            be2_sb = be2_all[:, l, :]
            bv_l = bv_d[l]
            bv_bc = small.tile([128, D], dt.float32, tag="bv", name=f"bv{l}")
            nc.gpsimd.dma_start(
                out=bv_bc[:, :],
                in_=bass.AP(tensor=bv_l.tensor, offset=bv_l.offset,
                            ap=[[0, 128]] + list(bv_l.ap)))

            # ---- K/V projections; a combined [K|V] AllGather is kicked per
            # feature chunk (= head pair) so attention chunk c can start as
            # soon as its own small gather lands, and the 8 gathers pipeline
            # under the whole attention phase. ----
            # K is feature-major [1024, NT]; V is token-major [NT, 1024].
            k_bf = qkvp.tile([128, CH, NT], dt.bfloat16, tag="k", name=f"k{l}")
            v_bf = qkvp.tile([128, TC, D], dt.bfloat16, tag="v", name=f"v{l}")

            def k_half_w(hf):
                wm = wpool.tile([128, CH, 512], dt.bfloat16, tag="w",
                                name=f"wk{l}_{hf}")
                nc.sync.dma_start(out=wm[:, :, :],
                                  in_=wk_r[:, :, hf * 512:(hf + 1) * 512])
                return wm

            def k_chunk(m, wm):
                ps = psum.tile([128, NT], dt.float32, tag="acc",
                               name=f"psk{l}_{m}")
                for kc in range(CH):
                    nc.tensor.matmul(
                        ps[:, :],
                        lhsT=wm[:, kc, (m % 4) * 128:(m % 4 + 1) * 128],
                        rhs=x_bf[:, kc, :],
                        start=(kc == 0), stop=(kc == CH - 1))
                nc.vector.tensor_scalar(out=k_bf[:, m, :], in0=ps[:, :],
                                        scalar1=bk_sb[:, m:m + 1],
                                        scalar2=None, op0=Alu.add)

            def v_half(hf):
                wvn = wpool.tile([128, CH, 512], dt.bfloat16, tag="w",
                                 name=f"wv{l}_{hf}")
                nc.sync.dma_start(out=wvn[:, :, :],
                                  in_=wv_r[:, :, hf * 512:(hf + 1) * 512])
                for tc4 in range(TC):
                    ps = psum.tile([128, 512], dt.float32, tag="acc",
                                   name=f"psv{l}_{hf}_{tc4}")
                    for kc in range(CH):
                        nc.tensor.matmul(
                            ps[:, :],
                            lhsT=x_bf[:, kc, tc4 * 128:(tc4 + 1) * 128],
                            rhs=wvn[:, kc, :],
                            start=(kc == 0), stop=(kc == CH - 1))
                    nc.vector.tensor_tensor(
                        out=v_bf[:, tc4, hf * 512:(hf + 1) * 512], in0=ps[:, :],
                        in1=bv_bc[:, hf * 512:(hf + 1) * 512], op=Alu.add)

            v_bf_hsplit = v_bf[:, :, :].rearrange("p t (c a b) -> p t c a b",
                                                  a=2, b=64)

            def kv_gather(c):
                own = own_kv[c]
                nc.scalar.dma_start(
                    out=own[0:KSZ].rearrange("(p q) -> p q", p=128),
                    in_=k_bf[:, c, :])
                ov = own[KSZ:].rearrange("(p kl a b) -> p kl a b", p=128,
                                         kl=TC, a=2, b=65)
                for i in range(2):
                    nc.gpsimd.dma_start(out=ov[:, :, i, 0:64],
                                        in_=v_bf_hsplit[:, :, c, i, :])
                nc.gpsimd.collective_compute(
                    "AllGather", Alu.bypass, replica_groups=groups,
                    ins=[own[:].opt()], outs=[gth_kv[c][:, :].opt()])

            wmk = k_half_w(0)
            for m in range(4):
                k_chunk(m, wmk)
            v_half(0)
            for c in range(4):
                kv_gather(c)
            wmk = k_half_w(1)
            for m in range(4, CH):
                k_chunk(m, wmk)
            v_half(1)
            for c in range(4, CH):
                kv_gather(c)

            # ---- Q projection (feature-major; head pair per chunk) ----
            q_bf = qkvp.tile([128, CH, NT], dt.bfloat16, tag="q", name=f"q{l}")
            for hf in range(2):
                wm = wpool.tile([128, CH, 512], dt.bfloat16, tag="w",
                                name=f"wq{l}_{hf}")
                nc.sync.dma_start(out=wm[:, :, :],
                                  in_=wq_r[:, :, hf * 512:(hf + 1) * 512])
                for m in range(4 * hf, 4 * hf + 4):
                    ps = psum.tile([128, NT], dt.float32, tag="acc",
                                   name=f"psq{l}_{m}")
                    for kc in range(CH):
                        nc.tensor.matmul(
                            ps[:, :],
                            lhsT=wm[:, kc, (m % 4) * 128:(m % 4 + 1) * 128],
                            rhs=x_bf[:, kc, :],
                            start=(kc == 0), stop=(kc == CH - 1))
                    nc.vector.tensor_scalar(out=q_bf[:, m, :], in0=ps[:, :],
                                            scalar1=bq_sb[:, m:m + 1],
                                            scalar2=None, op0=Alu.add)

            # ---- attention: one head PAIR per feature chunk; the two heads'
            # scores matmuls row-pack the PE array (K=64 each). The kc loop
            # runs scores+exp LAG steps ahead of the AV accumulation so the
            # Scalar exp stream (the phase's rate limiter) never waits on the
            # PE and vice versa. Wo runs as two half-passes interleaved into
            # the stream as PE filler, accumulating into x1a / x1. ----
            LAG = 4
            o_nbf = opool.tile([128, CH, NT], dt.bfloat16, tag="onb",
                               name=f"onb{l}")
            x1a = xres.tile([128, CH, NT], dt.bfloat16, tag="x1a", bufs=1,
                            name=f"x1a_{l}")
            x1 = xres.tile([128, CH, NT], dt.float32, tag="x", name=f"x1_{l}")
            filler = []

            def emit_filler(n):
                for _ in range(min(n, len(filler))):
                    filler.pop(0)()

            def wo_pass(half, wo_sb):
                # one output chunk m per filler slot: 4 accumulation matmuls
                # + one DVE op folding into x1a (pass A) or x1 (pass B)
                def mk(m):
                    def th():
                        ps = psum.tile([128, NT], dt.float32, tag="acc",
                                       name=f"pso{l}_{half}_{m}")
                        for kc in range(4):
                            nc.tensor.matmul(
                                ps[:, :], lhsT=wo_sb[:, kc, m * 128:(m + 1) * 128],
                                rhs=o_nbf[:, 4 * half + kc, :],
                                start=(kc == 0), stop=(kc == 3))
                        if half == 0:
                            nc.vector.tensor_scalar(out=x1a[:, m, :],
                                                    in0=ps[:, :],
                                                    scalar1=bo_sb[:, m:m + 1],
                                                    scalar2=None, op0=Alu.add)
                        else:
                            nc.vector.tensor_tensor(out=x1[:, m, :],
                                                    in0=ps[:, :],
                                                    in1=x1a[:, m, :],
                                                    op=Alu.add)
                            nc.vector.tensor_tensor(out=x1[:, m, :],
                                                    in0=x1[:, m, :],
                                                    in1=x[:, m, :], op=Alu.add)
                    return th
                for m in range(CH):
                    filler.append(mk(m))

            wo_sbs = []
            for c in range(CH):
                gv = gth_kv[c]
                k_sb = kvsp.tile([128, 4, NT], dt.bfloat16, tag="k",
                                 name=f"ksb{l}_{c}")
                nc.sync.dma_start(
                    out=k_sb[:, :, :],
                    in_=gv[:, 0:KSZ].rearrange("g (p q) -> p g q", p=128))
                v_sb = kvsp.tile([128, 4, TC, 130], dt.bfloat16, tag="v",
                                 name=f"vsb{l}_{c}")
                nc.gpsimd.dma_start(
                    out=v_sb[:, :, :, :],
                    in_=gv[:, KSZ:].rearrange("g (p kl f) -> p g kl f",
                                              p=128, f=130))
                avs = [psum.tile([128, NT], dt.float32, tag="av",
                                 name=f"av{l}_{2 * c + i}") for i in range(2)]
                ats = {}
                for kc in range(KCH + LAG):
                    if kc >= LAG:
                        j = kc - LAG
                        at = ats.pop(j)
                        for i in range(2):
                            nc.tensor.matmul(
                                avs[i][0:65, :],
                                lhsT=v_sb[:, j // 4, j % 4,
                                          65 * i:65 * i + 65],
                                rhs=at[:, i, :], start=(j == 0),
                                stop=(j == KCH - 1),
                                skip_group_check=True)
                        emit_filler(1)
                    if kc < KCH:
                        sc = psum.tile([128, 2, NT], dt.float32, tag="sc",
                                       name=f"sc{l}_{c}_{kc}")
                        for i in range(2):
                            nc.tensor.matmul(
                                sc[:, i, :],
                                lhsT=k_sb[64 * i:64 * i + 64, kc // 4,
                                          (kc % 4) * 128:(kc % 4 + 1) * 128],
                                rhs=q_bf[64 * i:64 * i + 64, c, :],
                                start=True, stop=True,
                                tile_position=(64 * i, 0))
                        at = attnp.tile([128, 2, NT], dt.bfloat16, tag="at",
                                        bufs=6, name=f"at{l}_{c}_{kc}")
                        nc.scalar.activation(out=at[:, :, :],
                                             in_=sc[:, :, :],
                                             func=Act.Exp, bias=0.0,
                                             scale=0.125)
                        ats[kc] = at
                # normalize by the softmax denominator (row 64 of av); this
                # executes under the next chunk's scores/exp stream.
                for i in range(2):
                    h = 2 * c + i
                    av = avs[i]
                    o_h = attnp.tile([64, NT], dt.bfloat16, tag="oh", bufs=2,
                                     name=f"oh{l}_{h}")
                    nc.vector.tensor_copy(out=o_h[:, :], in_=av[0:64, :])
                    z_h = stats.tile([1, NT], dt.float32, tag="zh",
                                     name=f"zh{l}_{h}")
                    nc.vector.tensor_copy(out=z_h[:, :], in_=av[64:65, :])
                    zi_f = stats.tile([1, NT], dt.float32, tag="zif",
                                      name=f"zif{l}_{h}")
                    nc.vector.reciprocal_approx_fast(out=zi_f[:, :],
                                                     in_=z_h[:, :])
                    zi_h = stats.tile([1, NT], dt.bfloat16, tag="zi",
                                      name=f"zi{l}_{h}")
                    with nc.allow_low_precision(reason="1/Z feeds a bf16 "
                                                "broadcast matmul anyway"):
                        nc.vector.tensor_copy(out=zi_h[:, :], in_=zi_f[:, :])
                    bz = psum.tile([64, NT], dt.float32, tag="acc",
                                   name=f"bz{l}_{h}")
                    nc.tensor.matmul(bz[:, :], lhsT=ones_row_bf[0:1, 0:64],
                                     rhs=zi_h[:, :], start=True, stop=True)
                    nc.vector.tensor_tensor(out=o_nbf[64 * i:64 * i + 64, c, :],
                                            in0=o_h[:, :], in1=bz[:, :],
                                            op=Alu.mult)
                # prefetch the Wo weight half needed one chunk later, then
                # queue its 8 output-chunk thunks as PE filler
                if c in (1, 5):
                    half = (c - 1) // 4
                    wo_sb = wpool.tile([128, 4, D], dt.bfloat16, tag="w",
                                       name=f"wo{l}_{half}")
                    nc.sync.dma_start(out=wo_sb[:, :, :],
                                      in_=wo_r[:, 4 * half:4 * half + 4, :])
                    wo_sbs.append(wo_sb)
                if c in (3, 7):
                    wo_pass((c - 3) // 4, wo_sbs[(c - 3) // 4])
            emit_filler(len(filler))

            x1n, x1n_bf = _ln(ctx, nc, psum, xres, xbfp, stats, x1, g1_sb,
                              be1_sb, ones_col_bf, ones_row_bf, eps_sb,
                              f"l{l}a")

            # ---- FFN ----
            x2 = xres.tile([128, CH, NT], dt.float32, tag="x", name=f"x2_{l}")
            for half in range(2):
                h_bf = hpool.tile([128, FCH // 2, NT], dt.bfloat16, tag="h",
                                  name=f"h{l}_{half}")
                for mi in range(FCH // 2):
                    m = half * (FCH // 2) + mi
                    w1m = wpool.tile([128, CH, 128], dt.bfloat16, tag="w",
                                     name=f"w1_{l}_{m}")
                    nc.sync.dma_start(out=w1m[:, :, :],
                                      in_=w1_r[:, :, m * 128:(m + 1) * 128])
                    ps = psum.tile([128, NT], dt.float32, tag="acc",
                                   name=f"ps1{l}_{m}")
                    for kc in range(CH):
                        nc.tensor.matmul(ps[:, :], lhsT=w1m[:, kc, :],
                                         rhs=x1n_bf[:, kc, :],
                                         start=(kc == 0), stop=(kc == CH - 1))
                    nc.vector.tensor_scalar(out=h_bf[:, mi, :], in0=ps[:, :],
                                            scalar1=b1_sb[:, m:m + 1],
                                            scalar2=0.0, op0=Alu.add,
                                            op1=Alu.max)
                for m in range(CH):
                    w2m = wpool.tile([128, FCH // 2, 128], dt.bfloat16, tag="w",
                                     name=f"w2_{l}_{half}_{m}")
                    nc.sync.dma_start(
                        out=w2m[:, :, :],
                        in_=w2_r[:, half * (FCH // 2):(half + 1) * (FCH // 2),
                                 m * 128:(m + 1) * 128])
                    ps = psum.tile([128, NT], dt.float32, tag="acc",
                                   name=f"ps2{l}_{half}_{m}")
                    for kc in range(FCH // 2):
                        nc.tensor.matmul(ps[:, :], lhsT=w2m[:, kc, :],
                                         rhs=h_bf[:, kc, :],
                                         start=(kc == 0),
                                         stop=(kc == FCH // 2 - 1))
                    if half == 0:
                        nc.vector.tensor_scalar(out=x2[:, m, :], in0=ps[:, :],
                                                scalar1=b2_sb[:, m:m + 1],
                                                scalar2=None, op0=Alu.add)
                    else:
                        nc.vector.tensor_tensor(out=x2[:, m, :],
                                                in0=x2[:, m, :], in1=ps[:, :],
                                                op=Alu.add)
                        nc.vector.tensor_tensor(out=x2[:, m, :],
                                                in0=x2[:, m, :],
                                                in1=x1n[:, m, :], op=Alu.add)

            x, x_bf = _ln(ctx, nc, psum, xres, xbfp, stats, x2, g2_sb, be2_sb,
                          ones_col_bf, ones_row_bf, eps_sb, f"l{l}b")

        # ---------------- output ----------------
        nc.sync.dma_start(
            out=out_d[:, :].rearrange("(c p) q -> p c q", p=128),
            in_=x[:, :, :])

    nc.compile()
    return nc


_PROG = {}


def _get_prog(layers=L):
    if layers not in _PROG:
        _PROG[layers] = build_program(layers)
    return _PROG[layers]


def _host_inputs(inputs, layers=L):
    """Build the 8 per-core input maps from the full-model inputs."""
    bf16 = ml_dtypes.bfloat16
    f32 = np.float32
    tokens = np.asarray(inputs["tokens"])
    # positional encoding (constant)
    pos = np.arange(S)[:, None].astype(f32)
    freq = np.exp(np.arange(0, D, 2).astype(f32) * -(math.log(10000.0) / D))
    pe = np.zeros((S, D), dtype=f32)
    pe[:, 0::2] = np.sin(pos * freq)
    pe[:, 1::2] = np.cos(pos * freq)
    shared = {
        "emb": np.ascontiguousarray(np.asarray(inputs["emb"], dtype=f32)),
    }
    for nm in ("Wq", "Wk", "Wv", "Wo", "W1", "W2"):
        shared[nm] = np.ascontiguousarray(
            np.asarray(inputs[nm])[:layers].astype(bf16))
    for nm in ("bq", "bk", "bv", "bo", "b1", "b2", "g1", "be1", "g2", "be2"):
        shared[nm] = np.ascontiguousarray(
            np.asarray(inputs[nm])[:layers].astype(f32))

    in_maps = []
    for core in range(NC):
        b, lc = core // 4, core % 4
        m = dict(shared)
        m["tokens_c"] = np.ascontiguousarray(tokens[b, lc * NT:(lc + 1) * NT])
        m["pe_fm"] = np.ascontiguousarray(pe[lc * NT:(lc + 1) * NT, :].T)
        in_maps.append(m)
    return in_maps


def run(inputs, layers=L, trace=False):
    nc = _get_prog(layers)
    in_maps = _host_inputs(inputs, layers)
    last_err = None
    for attempt in range(3):
        try:
            res = run_bass_kernel_spmd(nc, in_maps, list(range(NC)),
                                       trace=trace)
            break
        except Exception as e:  # transient NRT device faults observed
            last_err = e
            if attempt == 2:
                raise
            import time
            time.sleep(5)
    out = np.zeros((B, S, D), dtype=np.float32)
    for core in range(NC):
        b, lc = core // 4, core % 4
        out[b, lc * NT:(lc + 1) * NT, :] = res.results[core]["out_fm"].T
    return out, res


def kernel(**inputs):
    out, _ = run(inputs)
    return out

